# revision 2
# baseline (speedup 1.0000x reference)
"""Trainium2 Bass kernel for nn_ExactAttention (block-diagonal sparse attention).

Reference computes dense softmax attention over [N,N] then masks to
block-diagonal segments (batch_seg is sorted).  Only the diagonal blocks
survive, so we compute segment-local attention only.

The reference subtracts the *global* max of Q@K^T before exp; softmax is
shift-invariant except through EPS=1e-8, whose effect is ~1e-8 relative
(denominators are O(100+)), far below fp32 noise, so we skip the max
entirely (max |dot| ~ 70 -> exp(70/sqrt(128)) ~ 450, no overflow).

Sharding: 32 segments -> 8 cores x 4 slots, each slot zero-padded to L rows.
Host pre-transposes Q,K to [D, rows] (contraction dim must live in
partitions) and appends a "ones" column to V: padded key rows are all-zero
including the ones column, so they contribute exactly 0 to both numerator
and denominator — exact masking for free.  The denominator is just the
last column of the P @ [V|1] matmul.

Per core, per slot (L=512, nkc=L/128 key chunks):
  T_c   [128k x L]  = K_c Q^T        (PE; lhsT=K^T chunk, rhs=Q^T)
  P_c   [128k x L]  = exp(T_c/sqrt(d))  (ACT, straight from PSUM)
  O_qt  [128q x 129] = sum_c P_c[:,qt]^T @ [V_c|1]   (PE accumulate)
  out   = O[:, :128] * 1/(O[:,128]+eps)  (DVE reciprocal + per-partition mul)
"""

import numpy as np

import concourse.bass as bass
import concourse.mybir as mybir
import concourse.tile as tile
from concourse import bacc
from concourse import bass_utils

D = 128
N_CORES = 8
EPS = 1e-8
F32 = mybir.dt.float32

_program_cache = {}


def _build_program(n_slots: int, L: int):
    """Build + compile the SPMD program for n_slots segments of padded length L."""
    key = (n_slots, L)
    if key in _program_cache:
        return _program_cache[key]

    nkc = L // 128          # key chunks per slot
    nqt = L // 128          # query tiles per slot
    R = n_slots * L         # rows per core
    scale = float(1.0 / np.sqrt(np.float32(D)))

    nc = bacc.Bacc("TRN2", target_bir_lowering=False, debug=False,
                   num_devices=N_CORES)

    qt_d = nc.dram_tensor("qt", [D, R], F32, kind="ExternalInput").ap()
    kt_d = nc.dram_tensor("kt", [D, R], F32, kind="ExternalInput").ap()
    vx_d = nc.dram_tensor("vx", [D, n_slots * nkc * 129], F32,
                          kind="ExternalInput").ap()
    out_d = nc.dram_tensor("out", [R, D], F32, kind="ExternalOutput").ap()

    with tile.TileContext(nc) as tc:
        with tc.tile_pool(name="qk", bufs=4) as qk_pool, \
             tc.tile_pool(name="v", bufs=2) as v_pool, \
             tc.tile_pool(name="p", bufs=2 * nkc) as p_pool, \
             tc.tile_pool(name="small", bufs=8) as small_pool, \
             tc.tile_pool(name="osb", bufs=4) as o_pool, \
             tc.tile_pool(name="tps", bufs=2, space="PSUM") as t_psum, \
             tc.tile_pool(name="ops", bufs=4, space="PSUM") as o_psum:

            for s in range(n_slots):
                qs = qk_pool.tile([D, L], F32, tag="q")
                ks = qk_pool.tile([D, L], F32, tag="k")
                vs = v_pool.tile([D, nkc * 129], F32, tag="v")
                nc.sync.dma_start(qs[:], qt_d[:, s * L:(s + 1) * L])
                nc.sync.dma_start(ks[:], kt_d[:, s * L:(s + 1) * L])
                nc.sync.dma_start(vs[:], vx_d[:, s * nkc * 129:(s + 1) * nkc * 129])

                p_tiles = []
                for c in range(nkc):
                    t_ps = t_psum.tile([128, L], F32)
                    nc.tensor.matmul(t_ps[:], ks[:, c * 128:(c + 1) * 128], qs[:],
                                     start=True, stop=True)
                    p_sb = p_pool.tile([128, L], F32, tag="p")
                    nc.scalar.activation(p_sb[:], t_ps[:],
                                         mybir.ActivationFunctionType.Exp,
                                         scale=scale)
                    p_tiles.append(p_sb)

                for q in range(nqt):
                    o_ps = o_psum.tile([128, 129], F32)
                    for c in range(nkc):
                        nc.tensor.matmul(o_ps[:],
                                         p_tiles[c][:, q * 128:(q + 1) * 128],
                                         vs[:, c * 129:(c + 1) * 129],
                                         start=(c == 0), stop=(c == nkc - 1))
                    rec = small_pool.tile([128, 1], F32, tag="rec")
                    nc.vector.tensor_scalar_add(rec[:], o_ps[:, 128:129], EPS)
                    nc.vector.reciprocal(rec[:], rec[:])
                    o_sb = o_pool.tile([128, D], F32, tag="o")
                    nc.vector.tensor_scalar_mul(o_sb[:], o_ps[:, 0:128], rec[:])
                    nc.sync.dma_start(
                        out_d[s * L + q * 128: s * L + (q + 1) * 128, :], o_sb[:])

    nc.compile()
    _program_cache[key] = nc
    return nc


def kernel(Q, K, V, num_batch, batch_seg):
    Q = np.asarray(Q, dtype=np.float32)
    K = np.asarray(K, dtype=np.float32)
    V = np.asarray(V, dtype=np.float32)
    batch_seg = np.asarray(batch_seg)
    N = Q.shape[0]
    nb = int(num_batch)

    counts = np.bincount(batch_seg.astype(np.int64), minlength=nb)
    starts = np.zeros(nb + 1, dtype=np.int64)
    np.cumsum(counts, out=starts[1:])

    n_slots = (nb + N_CORES - 1) // N_CORES
    maxlen = int(counts.max()) if nb > 0 else 1
    L = max(128, -(-maxlen // 128) * 128)
    nkc = L // 128
    R = n_slots * L

    nc = _build_program(n_slots, L)

    # host-side shard prep: seg -> (core = seg // n_slots, slot = seg % n_slots)
    in_maps = []
    for core in range(N_CORES):
        Qp = np.zeros((R, D), np.float32)
        Kp = np.zeros((R, D), np.float32)
        Ve = np.zeros((R, 129), np.float32)
        for slot in range(n_slots):
            seg = core * n_slots + slot
            if seg >= nb:
                break
            b0, b1 = starts[seg], starts[seg + 1]
            ln = b1 - b0
            if ln == 0:
                continue
            Qp[slot * L: slot * L + ln] = Q[b0:b1]
            Kp[slot * L: slot * L + ln] = K[b0:b1]
            Ve[slot * L: slot * L + ln, :128] = V[b0:b1]
            Ve[slot * L: slot * L + ln, 128] = 1.0
        vh = np.ascontiguousarray(
            Ve.reshape(n_slots * nkc, 128, 129).transpose(1, 0, 2)
        ).reshape(D, n_slots * nkc * 129)
        in_maps.append({
            "qt": np.ascontiguousarray(Qp.T),
            "kt": np.ascontiguousarray(Kp.T),
            "vx": vh,
        })

    global _last_in_maps
    _last_in_maps = in_maps
    res = bass_utils.run_bass_kernel_spmd(nc, in_maps,
                                          core_ids=list(range(N_CORES)))

    out = np.empty((N, D), np.float32)
    for seg in range(nb):
        core, slot = seg // n_slots, seg % n_slots
        b0, b1 = starts[seg], starts[seg + 1]
        ln = b1 - b0
        if ln > 0:
            out[b0:b1] = res.results[core]["out"][slot * L: slot * L + ln]
    return out


# revision 3
# speedup vs baseline: 1.0468x; 1.0468x over previous
"""Trainium2 Bass kernel for nn_ExactAttention (block-diagonal sparse attention).

Reference computes dense softmax attention over [N,N] then masks to
block-diagonal segments (batch_seg is sorted).  Only the diagonal blocks
survive, so we compute segment-local attention only.

The reference subtracts the *global* max of Q@K^T before exp; softmax is
shift-invariant except through EPS=1e-8, whose effect is ~1e-8 relative
(denominators are O(100+)), far below fp32 noise, so we skip the max
entirely (max |dot| ~ 70 -> exp(70/sqrt(128)) ~ 450, no overflow).

Sharding: segments are sorted by length (desc) and dealt round-robin:
slot j of every core gets one of ranks [8j, 8j+8), all padded to the
group max L_j, so all 8 cores run one SPMD program with near-zero
padding waste and balanced work.

Host pre-transposes Q,K to [D, rows] (contraction dim must live in
partitions) and appends a "ones" column to V: padded key rows are all-zero
including the ones column, so they contribute exactly 0 to both numerator
and denominator — exact masking for free.  The denominator is the last
column of the P @ [V|1] matmul; the final division happens on host.

Per core, per slot (L rows, nkc=ceil(L/128) key chunks):
  T_c   [ck x L]    = K_c Q^T            (PE; lhsT=K^T chunk, rhs=Q^T)
  P_c   [ck x L]    = exp(T_c/sqrt(d))   (ACT, straight from PSUM)
  O_qt  [qk x 129]  = sum_c P_c[:,qt]^T @ [V_c|1]   (PE accumulate)
  out[R,129] -> host: out[:, :128] / (out[:, 128] + eps)
"""

import numpy as np

import concourse.bass as bass
import concourse.mybir as mybir
import concourse.tile as tile
from concourse import bacc
from concourse import bass_utils

D = 128
N_CORES = 8
EPS = 1e-8
F32 = mybir.dt.float32

_program_cache = {}


def _build_program(slot_lens):
    """Build + compile the SPMD program for per-slot padded lengths."""
    key = tuple(slot_lens)
    if key in _program_cache:
        return _program_cache[key]

    scale = float(1.0 / np.sqrt(np.float32(D)))
    R = sum(slot_lens)
    offs = np.concatenate([[0], np.cumsum(slot_lens)]).astype(int)
    nkcs = [(L + 127) // 128 for L in slot_lens]
    choffs = np.concatenate([[0], np.cumsum(nkcs)]).astype(int)
    C = int(choffs[-1])
    max_nkc = max(nkcs)

    nc = bacc.Bacc("TRN2", target_bir_lowering=False, debug=False,
                   num_devices=N_CORES)

    qt_d = nc.dram_tensor("qt", [D, R], F32, kind="ExternalInput").ap()
    kt_d = nc.dram_tensor("kt", [D, R], F32, kind="ExternalInput").ap()
    vx_d = nc.dram_tensor("vx", [D, C * 129], F32, kind="ExternalInput").ap()
    out_d = nc.dram_tensor("out", [R, 129], F32, kind="ExternalOutput").ap()

    with tile.TileContext(nc) as tc:
        with tc.tile_pool(name="qk", bufs=2) as qk_pool, \
             tc.tile_pool(name="v", bufs=2) as v_pool, \
             tc.tile_pool(name="p", bufs=2 * max_nkc) as p_pool, \
             tc.tile_pool(name="osb", bufs=6) as o_pool, \
             tc.tile_pool(name="tps", bufs=3, space="PSUM") as t_psum, \
             tc.tile_pool(name="ops", bufs=4, space="PSUM") as o_psum:

            for s, L in enumerate(slot_lens):
                nkc = nkcs[s]
                o0 = int(offs[s])
                c0 = int(choffs[s])
                ks = qk_pool.tile([D, L], F32, tag="k")
                qs = qk_pool.tile([D, L], F32, tag="q")
                vs = v_pool.tile([D, nkc * 129], F32, tag="v")
                nc.sync.dma_start(ks[:], kt_d[:, o0:o0 + L])
                nc.sync.dma_start(qs[:], qt_d[:, o0:o0 + L])
                nc.sync.dma_start(vs[:], vx_d[:, c0 * 129:(c0 + nkc) * 129])

                # query blocks of <=512 (PSUM bank limit / moving-max)
                for qb0 in range(0, L, 512):
                    qbs = min(512, L - qb0)
                    p_tiles = []
                    for c in range(nkc):
                        ck = min(128, L - c * 128)
                        t_ps = t_psum.tile([128, qbs], F32, tag="t")
                        nc.tensor.matmul(t_ps[:ck, :],
                                         ks[:, c * 128:c * 128 + ck],
                                         qs[:, qb0:qb0 + qbs],
                                         start=True, stop=True)
                        p_sb = p_pool.tile([128, qbs], F32, tag="p")
                        nc.scalar.activation(p_sb[:ck, :], t_ps[:ck, :],
                                             mybir.ActivationFunctionType.Exp,
                                             scale=scale)
                        p_tiles.append(p_sb)

                    for q0 in range(0, qbs, 128):
                        qk = min(128, qbs - q0)
                        o_ps = o_psum.tile([128, 129], F32, tag="ops")
                        for c in range(nkc):
                            ck = min(128, L - c * 128)
                            nc.tensor.matmul(o_ps[:qk, :],
                                             p_tiles[c][:ck, q0:q0 + qk],
                                             vs[:ck, c * 129:(c + 1) * 129],
                                             start=(c == 0), stop=(c == nkc - 1))
                        o_sb = o_pool.tile([128, 129], F32, tag="o")
                        nc.vector.tensor_copy(o_sb[:qk, :], o_ps[:qk, :])
                        r0 = o0 + qb0 + q0
                        nc.sync.dma_start(out_d[r0:r0 + qk, :], o_sb[:qk, :])

    nc.compile()
    _program_cache[key] = nc
    return nc


def kernel(Q, K, V, num_batch, batch_seg):
    Q = np.asarray(Q, dtype=np.float32)
    K = np.asarray(K, dtype=np.float32)
    V = np.asarray(V, dtype=np.float32)
    batch_seg = np.asarray(batch_seg)
    N = Q.shape[0]
    nb = int(num_batch)

    counts = np.bincount(batch_seg.astype(np.int64), minlength=nb)
    starts = np.zeros(nb + 1, dtype=np.int64)
    np.cumsum(counts, out=starts[1:])

    # rank segments by length desc; slot j <- ranks [8j, 8j+8)
    order = np.argsort(-counts, kind="stable")
    n_slots = (nb + N_CORES - 1) // N_CORES
    slot_lens = []
    assign = {}  # (core, slot) -> seg id
    for j in range(n_slots):
        grp = order[j * N_CORES:(j + 1) * N_CORES]
        slot_lens.append(max(1, int(counts[grp].max())))
        for c, seg in enumerate(grp):
            assign[(c, j)] = int(seg)

    offs = np.concatenate([[0], np.cumsum(slot_lens)]).astype(int)
    nkcs = [(L + 127) // 128 for L in slot_lens]
    choffs = np.concatenate([[0], np.cumsum(nkcs)]).astype(int)
    R = int(offs[-1])
    C = int(choffs[-1])

    nc = _build_program(tuple(slot_lens))

    in_maps = []
    for core in range(N_CORES):
        Qp = np.zeros((R, D), np.float32)
        Kp = np.zeros((R, D), np.float32)
        Ve = np.zeros((C * 128, 129), np.float32)
        for j in range(n_slots):
            seg = assign.get((core, j))
            if seg is None:
                continue
            b0, b1 = starts[seg], starts[seg + 1]
            ln = int(b1 - b0)
            if ln == 0:
                continue
            o0 = int(offs[j])
            Qp[o0:o0 + ln] = Q[b0:b1]
            Kp[o0:o0 + ln] = K[b0:b1]
            v0 = int(choffs[j]) * 128
            Ve[v0:v0 + ln, :128] = V[b0:b1]
            Ve[v0:v0 + ln, 128] = 1.0
        vh = np.ascontiguousarray(
            Ve.reshape(C, 128, 129).transpose(1, 0, 2)).reshape(D, C * 129)
        in_maps.append({
            "qt": np.ascontiguousarray(Qp.T),
            "kt": np.ascontiguousarray(Kp.T),
            "vx": vh,
        })

    global _last_in_maps
    _last_in_maps = in_maps
    res = bass_utils.run_bass_kernel_spmd(nc, in_maps,
                                          core_ids=list(range(N_CORES)))

    out = np.empty((N, D), np.float32)
    for (core, j), seg in assign.items():
        b0, b1 = starts[seg], starts[seg + 1]
        ln = int(b1 - b0)
        if ln == 0:
            continue
        o0 = int(offs[j])
        raw = res.results[core]["out"][o0:o0 + ln]
        out[b0:b1] = raw[:, :128] / (raw[:, 128:129] + EPS)
    return out


# revision 5
# speedup vs baseline: 1.2195x; 1.1649x over previous
"""Trainium2 Bass kernel for nn_ExactAttention (block-diagonal sparse attention).

Reference computes dense softmax attention over [N,N] then masks to
block-diagonal segments (batch_seg is sorted).  Only the diagonal blocks
survive, so we compute segment-local attention only.

The reference subtracts the *global* max of Q@K^T before exp; softmax is
shift-invariant except through EPS=1e-8, whose effect is ~1e-8 relative
(denominators are O(100+)), far below fp32 noise, so we skip the max
entirely (max |dot| ~ 70 -> exp(70/sqrt(128)) ~ 450, no overflow).

Sharding: segments are sorted by length (desc) and dealt round-robin:
slot j of every core gets one of ranks [8j, 8j+8), all padded to the
group max L_j, so all 8 cores run one SPMD program with near-zero
padding waste and balanced work.

Layout (all chosen to keep the PE matmul-cycle count minimal in fp32):
  scores:  T_c [ck x m]  = K_c Q^T        (lhsT = K^T chunk, rhs = Q^T)
  exp:     P_c [ck x m]  = exp(T_c/sqrt(d))      (ACT, from PSUM)
  AV:      O^T [128 x m] += V_c^T P_c     (lhsT = V_c natural layout!,
                                           rhs = P_c — V-stationary keeps
                                           weight loads at one per chunk)
  den:     S [128 x m] = sum_c P_c        (DVE adds; host sums partitions)
Host divides O^T.T by (den + eps) and scatters rows back.  Padded key
rows have V=0 and P=exp(0)=1 but are excluded on host because den is
computed... padded keys DO contribute exp(0)=1 to S!  -> handled by
masking on host?  No: padded rows of Q/K are zero so T=0, exp(0)*V0=0
for the numerator, but S would include them.  We therefore subtract the
pad contribution on host: pad keys contribute exactly (L - len) * 1.0
... EXCEPT T[pad_k, q] = 0 only if BOTH q and k pads are zero — they
are (host zero-pads).  So den_true = S_hostsum - (L - len).
"""

import numpy as np

import concourse.bass as bass
import concourse.mybir as mybir
import concourse.tile as tile
from concourse import bacc
from concourse import bass_utils

D = 128
N_CORES = 8
EPS = 1e-8
F32 = mybir.dt.float32

_program_cache = {}


def _build_program(slot_lens):
    """Build + compile the SPMD program for per-slot padded lengths."""
    key = tuple(slot_lens)
    if key in _program_cache:
        return _program_cache[key]

    scale = float(1.0 / np.sqrt(np.float32(D)))
    R = sum(slot_lens)
    offs = np.concatenate([[0], np.cumsum(slot_lens)]).astype(int)
    nkcs = [(L + 127) // 128 for L in slot_lens]
    choffs = np.concatenate([[0], np.cumsum(nkcs)]).astype(int)
    C = int(choffs[-1])
    max_nkc = max(nkcs)

    nc = bacc.Bacc("TRN2", target_bir_lowering=False, debug=False,
                   num_devices=N_CORES)

    qt_d = nc.dram_tensor("qt", [D, R], F32, kind="ExternalInput").ap()
    kt_d = nc.dram_tensor("kt", [D, R], F32, kind="ExternalInput").ap()
    vx_d = nc.dram_tensor("vx", [D, C * 128], F32, kind="ExternalInput").ap()
    ot_d = nc.dram_tensor("ot", [D, R], F32, kind="ExternalOutput").ap()
    s_d = nc.dram_tensor("s", [D, R], F32, kind="ExternalOutput").ap()

    with tile.TileContext(nc) as tc:
        with tc.tile_pool(name="qk", bufs=2) as qk_pool, \
             tc.tile_pool(name="v", bufs=2) as v_pool, \
             tc.tile_pool(name="p", bufs=2 * max_nkc) as p_pool, \
             tc.tile_pool(name="sums", bufs=2) as s_pool, \
             tc.tile_pool(name="osb", bufs=2) as o_pool, \
             tc.tile_pool(name="tps", bufs=3, space="PSUM") as t_psum, \
             tc.tile_pool(name="ops", bufs=2, space="PSUM") as o_psum:

            for s, L in enumerate(slot_lens):
                nkc = nkcs[s]
                o0 = int(offs[s])
                c0 = int(choffs[s])
                ks = qk_pool.tile([D, L], F32, tag="k")
                qs = qk_pool.tile([D, L], F32, tag="q")
                vs = v_pool.tile([D, nkc * 128], F32, tag="v")
                nc.sync.dma_start(ks[:], kt_d[:, o0:o0 + L])
                nc.sync.dma_start(qs[:], qt_d[:, o0:o0 + L])
                nc.sync.dma_start(vs[:], vx_d[:, c0 * 128:(c0 + nkc) * 128])

                # query blocks of <=512 (PSUM bank limit / moving-max)
                for qb0 in range(0, L, 512):
                    qbs = min(512, L - qb0)
                    p_tiles = []
                    for c in range(nkc):
                        ck = min(128, L - c * 128)
                        t_ps = t_psum.tile([128, qbs], F32, tag="t")
                        nc.tensor.matmul(t_ps[:ck, :],
                                         ks[:, c * 128:c * 128 + ck],
                                         qs[:, qb0:qb0 + qbs],
                                         start=True, stop=True)
                        p_sb = p_pool.tile([128, qbs], F32, tag="p")
                        nc.scalar.activation(p_sb[:ck, :], t_ps[:ck, :],
                                             mybir.ActivationFunctionType.Exp,
                                             scale=scale)
                        p_tiles.append(p_sb)

                    # AV: O^T += V_c^T P_c  (V stationary, one weight/chunk)
                    o_ps = o_psum.tile([128, qbs], F32, tag="ops")
                    for c in range(nkc):
                        ck = min(128, L - c * 128)
                        nc.tensor.matmul(o_ps[:],
                                         vs[:ck, c * 128:(c + 1) * 128],
                                         p_tiles[c][:ck, :],
                                         start=(c == 0), stop=(c == nkc - 1))

                    # den partials: S = sum_c P_c (DVE); host sums partitions.
                    # Only the valid [:ck] partitions of each P tile are
                    # written by exp; partial chunks are slice-added so stale
                    # partitions never leak into S.
                    s_sb = s_pool.tile([128, qbs], F32, tag="s")
                    ck0 = min(128, L)
                    if ck0 < 128:
                        nc.gpsimd.memset(s_sb[:], 0.0)
                        nc.vector.tensor_add(s_sb[:ck0, :], s_sb[:ck0, :],
                                             p_tiles[0][:ck0, :])
                    else:
                        nc.vector.tensor_copy(s_sb[:], p_tiles[0][:])
                    for c in range(1, nkc):
                        ck = min(128, L - c * 128)
                        nc.vector.tensor_add(s_sb[:ck, :], s_sb[:ck, :],
                                             p_tiles[c][:ck, :])
                    o_sb = o_pool.tile([128, qbs], F32, tag="o")
                    nc.vector.tensor_copy(o_sb[:], o_ps[:])
                    nc.sync.dma_start(ot_d[:, o0 + qb0:o0 + qb0 + qbs], o_sb[:])
                    nc.sync.dma_start(s_d[:, o0 + qb0:o0 + qb0 + qbs], s_sb[:])

    nc.compile()
    _program_cache[key] = nc
    return nc


def kernel(Q, K, V, num_batch, batch_seg):
    Q = np.asarray(Q, dtype=np.float32)
    K = np.asarray(K, dtype=np.float32)
    V = np.asarray(V, dtype=np.float32)
    batch_seg = np.asarray(batch_seg)
    N = Q.shape[0]
    nb = int(num_batch)

    counts = np.bincount(batch_seg.astype(np.int64), minlength=nb)
    starts = np.zeros(nb + 1, dtype=np.int64)
    np.cumsum(counts, out=starts[1:])

    # rank segments by length desc; slot j <- ranks [8j, 8j+8)
    order = np.argsort(-counts, kind="stable")
    n_slots = (nb + N_CORES - 1) // N_CORES
    slot_lens = []
    assign = {}  # (core, slot) -> seg id
    for j in range(n_slots):
        grp = order[j * N_CORES:(j + 1) * N_CORES]
        slot_lens.append(max(1, int(counts[grp].max())))
        for c, seg in enumerate(grp):
            assign[(c, j)] = int(seg)

    offs = np.concatenate([[0], np.cumsum(slot_lens)]).astype(int)
    nkcs = [(L + 127) // 128 for L in slot_lens]
    choffs = np.concatenate([[0], np.cumsum(nkcs)]).astype(int)
    R = int(offs[-1])
    C = int(choffs[-1])

    nc = _build_program(tuple(slot_lens))

    in_maps = []
    for core in range(N_CORES):
        Qp = np.zeros((R, D), np.float32)
        Kp = np.zeros((R, D), np.float32)
        Vp = np.zeros((C * 128, D), np.float32)
        for j in range(n_slots):
            seg = assign.get((core, j))
            if seg is None:
                continue
            b0, b1 = starts[seg], starts[seg + 1]
            ln = int(b1 - b0)
            if ln == 0:
                continue
            o0 = int(offs[j])
            Qp[o0:o0 + ln] = Q[b0:b1]
            Kp[o0:o0 + ln] = K[b0:b1]
            v0 = int(choffs[j]) * 128
            Vp[v0:v0 + ln] = V[b0:b1]
        vh = np.ascontiguousarray(
            Vp.reshape(C, 128, D).transpose(1, 0, 2)).reshape(D, C * 128)
        in_maps.append({
            "qt": np.ascontiguousarray(Qp.T),
            "kt": np.ascontiguousarray(Kp.T),
            "vx": vh,
        })

    global _last_in_maps
    _last_in_maps = in_maps
    res = bass_utils.run_bass_kernel_spmd(nc, in_maps,
                                          core_ids=list(range(N_CORES)))

    out = np.empty((N, D), np.float32)
    for (core, j), seg in assign.items():
        b0, b1 = starts[seg], starts[seg + 1]
        ln = int(b1 - b0)
        if ln == 0:
            continue
        o0 = int(offs[j])
        L = slot_lens[j]
        otT = res.results[core]["ot"][:, o0:o0 + ln]          # [D, ln]
        s = res.results[core]["s"][:, o0:o0 + ln]             # [128, ln]
        # padded keys contribute exp(0)=1 each to the raw column sums
        den = s.sum(axis=0, dtype=np.float64) - float(L - ln) + EPS
        out[b0:b1] = (otT.T / den[:, None]).astype(np.float32)
    return out


# revision 10
# speedup vs baseline: 1.2363x; 1.0138x over previous
"""Trainium2 Bass kernel for nn_ExactAttention (block-diagonal sparse attention).

Reference computes dense softmax attention over [N,N] then masks to
block-diagonal segments (batch_seg is sorted).  Only the diagonal blocks
survive, so we compute segment-local attention only.

The reference subtracts the *global* max of Q@K^T before exp; softmax is
shift-invariant except through EPS=1e-8, whose effect is ~1e-8 relative
(denominators are O(100+)), far below fp32 noise, so we skip the max
entirely (max |dot| ~ 70 -> exp(70/sqrt(128)) ~ 450, no overflow).

Sharding: segments are sorted by length (desc) and dealt round-robin:
slot j of every core gets one of ranks [8j, 8j+8), all padded to the
group max L_j, so all 8 cores run one SPMD program with near-zero
padding waste and balanced work.

Layout (all chosen to keep the PE matmul-cycle count minimal in fp32):
  scores:  T_c [ck x m]  = K_c Q^T        (lhsT = K^T chunk, rhs = Q^T)
  exp:     P_c [ck x m]  = exp(T_c/sqrt(d))      (ACT, from PSUM)
  AV:      O^T [128 x m] += V_c^T P_c     (lhsT = V_c natural layout!,
                                           rhs = P_c — V-stationary keeps
                                           weight loads at one per chunk)
  den:     S [128 x m] = sum_c P_c        (DVE adds; host sums partitions)
Host divides O^T.T by (den + eps) and scatters rows back.  Padded key
rows have V=0 and P=exp(0)=1 but are excluded on host because den is
computed... padded keys DO contribute exp(0)=1 to S!  -> handled by
masking on host?  No: padded rows of Q/K are zero so T=0, exp(0)*V0=0
for the numerator, but S would include them.  We therefore subtract the
pad contribution on host: pad keys contribute exactly (L - len) * 1.0
... EXCEPT T[pad_k, q] = 0 only if BOTH q and k pads are zero — they
are (host zero-pads).  So den_true = S_hostsum - (L - len).
"""

import numpy as np

import concourse.bass as bass
import concourse.mybir as mybir
import concourse.tile as tile
from concourse import bacc
from concourse import bass_utils

D = 128
N_CORES = 8
EPS = 1e-8
F32 = mybir.dt.float32

_program_cache = {}


def _build_program(slot_lens):
    """Build + compile the SPMD program for per-slot padded lengths."""
    key = tuple(slot_lens)
    if key in _program_cache:
        return _program_cache[key]

    scale = float(1.0 / np.sqrt(np.float32(D)))
    R = sum(slot_lens)
    offs = np.concatenate([[0], np.cumsum(slot_lens)]).astype(int)
    nkcs = [(L + 127) // 128 for L in slot_lens]
    choffs = np.concatenate([[0], np.cumsum(nkcs)]).astype(int)
    C = int(choffs[-1])
    max_nkc = max(nkcs)

    nc = bacc.Bacc("TRN2", target_bir_lowering=False, debug=False,
                   num_devices=N_CORES)

    qt_d = nc.dram_tensor("qt", [D, R], F32, kind="ExternalInput").ap()
    kt_d = nc.dram_tensor("kt", [D, R], F32, kind="ExternalInput").ap()
    vx_d = nc.dram_tensor("vx", [D, C * 128], F32, kind="ExternalInput").ap()
    # merged [O^T | S] output: slot j occupies columns [2*offs[j], 2*offs[j]+2L)
    os_d = nc.dram_tensor("os", [D, 2 * R], F32, kind="ExternalOutput").ap()

    with tile.TileContext(nc) as tc:
        with tc.tile_pool(name="qk", bufs=2) as qk_pool, \
             tc.tile_pool(name="v", bufs=2) as v_pool, \
             tc.tile_pool(name="p", bufs=2 * max_nkc) as p_pool, \
             tc.tile_pool(name="osb", bufs=2) as o_pool, \
             tc.tile_pool(name="tps", bufs=3, space="PSUM") as t_psum, \
             tc.tile_pool(name="ops", bufs=2, space="PSUM") as o_psum:

            for s, L in enumerate(slot_lens):
                nkc = nkcs[s]
                o0 = int(offs[s])
                c0 = int(choffs[s])
                ks = qk_pool.tile([D, L], F32, tag="k")
                qs = qk_pool.tile([D, L], F32, tag="q")
                vs = v_pool.tile([D, nkc * 128], F32, tag="v")
                # per-chunk K loads so the first score matmul starts as soon
                # as K chunk 0 + Q have landed; Q on the scalar HWDGE queue
                # to parallelize descriptor generation with sync.
                ck0 = min(128, L)
                nc.sync.dma_start(ks[:, :ck0], kt_d[:, o0:o0 + ck0])
                nc.scalar.dma_start(qs[:], qt_d[:, o0:o0 + L])
                if L > 128:
                    nc.sync.dma_start(ks[:, 128:], kt_d[:, o0 + 128:o0 + L])
                nc.sync.dma_start(vs[:], vx_d[:, c0 * 128:(c0 + nkc) * 128])

                # query blocks of <=512 (PSUM bank limit / moving-max)
                for qb0 in range(0, L, 512):
                    qbs = min(512, L - qb0)
                    p_tiles = []
                    for c in range(nkc):
                        ck = min(128, L - c * 128)
                        t_ps = t_psum.tile([128, qbs], F32, tag="t")
                        nc.tensor.matmul(t_ps[:ck, :],
                                         ks[:, c * 128:c * 128 + ck],
                                         qs[:, qb0:qb0 + qbs],
                                         start=True, stop=True)
                        p_sb = p_pool.tile([128, qbs], F32, tag="p")
                        nc.scalar.activation(p_sb[:ck, :], t_ps[:ck, :],
                                             mybir.ActivationFunctionType.Exp,
                                             scale=scale)
                        p_tiles.append(p_sb)

                    # AV: O^T += V_c^T P_c  (V stationary, one weight/chunk)
                    o_ps = o_psum.tile([128, qbs], F32, tag="ops")
                    for c in range(nkc):
                        ck = min(128, L - c * 128)
                        nc.tensor.matmul(o_ps[:],
                                         vs[:ck, c * 128:(c + 1) * 128],
                                         p_tiles[c][:ck, :],
                                         start=(c == 0), stop=(c == nkc - 1))

                    # Merged [O^T | S] tile: one store per slot.
                    # S = sum_c P_c (DVE); host sums partitions for den.
                    # Only the valid [:ck] partitions of each P tile are
                    # written by exp; partial chunks are slice-added so stale
                    # partitions never leak into S.
                    os_sb = o_pool.tile([128, 2 * qbs], F32, tag="o")
                    s_ap = os_sb[:, qbs:2 * qbs]
                    sck0 = min(128, L)
                    if sck0 < 128:
                        nc.gpsimd.memset(s_ap, 0.0)
                        nc.vector.tensor_add(s_ap[:sck0, :], s_ap[:sck0, :],
                                             p_tiles[0][:sck0, :])
                    else:
                        nc.vector.tensor_copy(s_ap, p_tiles[0][:])
                    for c in range(1, nkc):
                        ck = min(128, L - c * 128)
                        nc.vector.tensor_add(s_ap[:ck, :], s_ap[:ck, :],
                                             p_tiles[c][:ck, :])
                    nc.vector.tensor_copy(os_sb[:, :qbs], o_ps[:])
                    d0 = 2 * o0 + 2 * qb0
                    nc.sync.dma_start(os_d[:, d0:d0 + 2 * qbs], os_sb[:])

    nc.compile()
    _program_cache[key] = nc
    return nc


def kernel(Q, K, V, num_batch, batch_seg):
    Q = np.asarray(Q, dtype=np.float32)
    K = np.asarray(K, dtype=np.float32)
    V = np.asarray(V, dtype=np.float32)
    batch_seg = np.asarray(batch_seg)
    N = Q.shape[0]
    nb = int(num_batch)

    counts = np.bincount(batch_seg.astype(np.int64), minlength=nb)
    starts = np.zeros(nb + 1, dtype=np.int64)
    np.cumsum(counts, out=starts[1:])

    # rank segments by length desc; slot j <- ranks [8j, 8j+8)
    order = np.argsort(-counts, kind="stable")
    n_slots = (nb + N_CORES - 1) // N_CORES
    slot_lens = []
    assign = {}  # (core, slot) -> seg id
    for j in range(n_slots):
        grp = order[j * N_CORES:(j + 1) * N_CORES]
        slot_lens.append(max(1, int(counts[grp].max())))
        for c, seg in enumerate(grp):
            assign[(c, j)] = int(seg)

    offs = np.concatenate([[0], np.cumsum(slot_lens)]).astype(int)
    nkcs = [(L + 127) // 128 for L in slot_lens]
    choffs = np.concatenate([[0], np.cumsum(nkcs)]).astype(int)
    R = int(offs[-1])
    C = int(choffs[-1])

    nc = _build_program(tuple(slot_lens))

    in_maps = []
    for core in range(N_CORES):
        Qp = np.zeros((R, D), np.float32)
        Kp = np.zeros((R, D), np.float32)
        Vp = np.zeros((C * 128, D), np.float32)
        for j in range(n_slots):
            seg = assign.get((core, j))
            if seg is None:
                continue
            b0, b1 = starts[seg], starts[seg + 1]
            ln = int(b1 - b0)
            if ln == 0:
                continue
            o0 = int(offs[j])
            Qp[o0:o0 + ln] = Q[b0:b1]
            Kp[o0:o0 + ln] = K[b0:b1]
            v0 = int(choffs[j]) * 128
            Vp[v0:v0 + ln] = V[b0:b1]
        vh = np.ascontiguousarray(
            Vp.reshape(C, 128, D).transpose(1, 0, 2)).reshape(D, C * 128)
        in_maps.append({
            "qt": np.ascontiguousarray(Qp.T),
            "kt": np.ascontiguousarray(Kp.T),
            "vx": vh,
        })

    global _last_in_maps
    _last_in_maps = in_maps
    res = bass_utils.run_bass_kernel_spmd(nc, in_maps,
                                          core_ids=list(range(N_CORES)))

    out = np.empty((N, D), np.float32)
    for (core, j), seg in assign.items():
        b0, b1 = starts[seg], starts[seg + 1]
        ln = int(b1 - b0)
        if ln == 0:
            continue
        o0 = int(offs[j])
        L = slot_lens[j]
        osr = res.results[core]["os"]                       # [D, 2R]
        # unpack per-qblock [ot(qbs) | s(qbs)] layout
        otT = np.empty((D, L), np.float32)
        sS = np.empty((D, L), np.float32)
        for qb0 in range(0, L, 512):
            qbs = min(512, L - qb0)
            d0 = 2 * o0 + 2 * qb0
            otT[:, qb0:qb0 + qbs] = osr[:, d0:d0 + qbs]
            sS[:, qb0:qb0 + qbs] = osr[:, d0 + qbs:d0 + 2 * qbs]
        # padded keys contribute exp(0)=1 each to the raw column sums
        den = sS[:, :ln].sum(axis=0, dtype=np.float64) - float(L - ln) + EPS
        out[b0:b1] = (otT[:, :ln].T / den[:, None]).astype(np.float32)
    return out
